# revision 24
# baseline (speedup 1.0000x reference)
"""Trainium2 Bass kernel for nn_MultiHeadAttention_42923903156587.

Sharding: 8 cores = 2 batches x 4 head-groups. Core (b, g) computes attention
for batch b, heads [4g, 4g+4), processed as two head PAIRS stacked on the
128-partition axis (head A on partitions 0:64, head B on 64:128):

- q/k convs: per-pair stacked weights; A and B matmuls are row+col tiled
  (tile_position derived from base partitions) so they run concurrently.
  RoPE is applied by full-128-lane ops: (conv+bias)*cos_mask +
  (swap_conv+bias)*sin_mask, where the "swap" channels come from extra
  permuted-weight matmuls and the masks zero the non-rotated rows.
  The two mask multiplies run on DVE; the combining add runs on GPSIMD.
- v is produced transposed (t, c) directly by the conv (lhsT = c-activation
  tile), so the softmax 1/denom (per t) is a per-partition tensor_scalar.
- scores/out matmuls run in bf16 (inputs rounded; fp32 PSUM accumulate).
  Softmax skips max-subtraction (logits are O(1)); row sums come free from
  the exp ACTIVATE accum_out.
- The strip loop is software-pipelined: strip i's score matmuls are emitted
  BEFORE strip i-1's PV matmuls so the in-order PE queue never head-of-line
  blocks on the exp (Act engine) of the current strip. Score tiles are
  ordered hh-major so each strip needs only 2 weight loads.
- out = v @ attn accumulates in one (128, T) PSUM region, head A in
  partitions 0:64, head B in 64:128 via col tile_position. The PSUM->SBUF
  drain runs on the scalar engine (idle at pair end).
- An AllGather per head-pair (overlapped with the next pair's compute)
  assembles all heads; each core then runs the full conv_o on its quarter
  of the time axis (selected with cc_rank), transposed (out rows = t) so
  weight loads halve; wo rows host-permuted to match the gathered layout,
  output transposed back on the host.
"""
import math
from contextlib import ExitStack

import ml_dtypes
import numpy as np

import concourse.bass as bass
import concourse.mybir as mybir
import concourse.tile as tile
from concourse.bass_utils import run_bass_kernel_spmd

# ---------------------------------------------------------------------------
# Workaround for this walrus build: at most ONE embedded sync-wait per TPB
# instruction is accepted. Split multi-wait instructions into single-wait NOPs.
# ---------------------------------------------------------------------------
from concourse.tile import TileContext, ScopedClock

_orig_lower = TileContext._lower_ordered_insts
_uid = [0]


def _mknop(engine, waits):
    _uid[0] += 1
    n = mybir.InstNoOp(name=f"I-waitsplit-{_uid[0]}", bass_nofuse=True)
    n.engine = engine
    n.sync_info = mybir.SyncInfo(on_wait=list(waits), on_update=[])
    return n


def _split_waits_in_list(insts):
    out = []
    for inst in insts:
        si = getattr(inst, "sync_info", None)
        if si is not None and si.on_wait and len(si.on_wait) > 1:
            waits = list(si.on_wait)
            for w in waits[:-1]:
                out.append(_mknop(inst.engine, [w]))
            inst.sync_info = mybir.SyncInfo(
                on_wait=[waits[-1]],
                on_update=list(si.on_update) if si.on_update else [],
            )
        out.append(inst)
    return out


def _patched_lower(self, ordered):
    for name in list(ordered.keys()):
        ordered[name] = _split_waits_in_list(ordered[name])
    return _orig_lower(self, ordered)


def _patched_drain_and_barrier(self, tick_clock, wait_clock):
    nc = self.nc
    carrier = nc.sync.nop(nofuse=True)
    wait_clock.add_sem_waits(carrier.ins, ScopedClock({None: tick_clock.global_clock}))
    si = carrier.ins.sync_info
    waits = list(si.on_wait) if si is not None and si.on_wait else []
    if len(waits) > 1:
        carrier.ins.sync_info = mybir.SyncInfo(
            on_wait=waits[:1],
            on_update=list(si.on_update) if si.on_update else [],
        )
        for w in waits[1:]:
            extra = nc.sync.nop(nofuse=True)
            extra.ins.sync_info = mybir.SyncInfo(on_wait=[w], on_update=[])
    nc.sync.drain()

    nc.all_engine_barrier()
    assert self.sems is not None
    popped = nc._tile_sem_poison_stack.pop()
    assert popped is self._sem_poison
    nc.clear_and_free_semaphores(list(self.sems.allocated().values()))
    nc.all_engine_barrier()


TileContext._lower_ordered_insts = _patched_lower
TileContext._drain_and_barrier = _patched_drain_and_barrier

# ---------------------------------------------------------------------------

F32 = mybir.dt.float32
F32R = mybir.dt.float32r
BF16 = mybir.dt.bfloat16
AF = mybir.ActivationFunctionType
ALU = mybir.AluOpType

B, C, T, H = 2, 1024, 2048, 16
CK = 64          # head dim
ROT = 32         # rotated head dims
HALF = 16
C_OUT = 1024
HPC = 4          # heads per core
NP = 2           # head pairs per core
G = 4            # cores per batch group
TSL = T // G     # 512: o-proj T slice per core
NT = T // 128    # 16 t-tiles per head
SC = 1.0 / math.sqrt(CK)
GROUPS = [[0, 1, 2, 3], [4, 5, 6, 7]]


def build_nc():
    nc = bass.Bass("TRN2", target_bir_lowering=False, debug=False, num_devices=8)

    def P(name, shape, dtype, out=False):
        return nc.dram_tensor(name, list(shape), dtype,
                              kind="ExternalOutput" if out else "ExternalInput").ap()

    xh = P("xh", (HPC * CK, T), BF16)        # x channels, pair p rows 128p:128p+128
    chd = P("ch", (HPC * CK, T), BF16)
    # block-diagonal per-pair conv weights: one K=128 matmul per chunk
    qw = P("qw", (NP * 128, 128), BF16)      # blockdiag(wqA.T, wqB.T)
    qsw = P("qsw", (NP * 128, 128), BF16)    # perm weights in cols 0:32 / 64:96
    kw = P("kw", (NP * 128, 128), BF16)
    ksw = P("ksw", (NP * 128, 128), BF16)
    vw = P("vw", (NP * 128, 128), BF16)      # blockdiag(wvA.T, wvB.T)
    bqm = P("bqm", (NP * 128, 1), F32)       # [bqA; bqB]
    bqs = P("bqs", (NP * 128, 1), F32)       # [bqA[PERM]; 0; bqB[PERM]; 0]
    bkm = P("bkm", (NP * 128, 1), F32)
    bks = P("bks", (NP * 128, 1), F32)
    bvp = P("bvp", (NP, 8 * 128), F32)       # per pair: tile([bvA; bvB], 8)
    cmask = P("cmask", (128, T), F32)        # [cos; cos; 1; 1] x2
    smask = P("smask", (128, T), F32)        # [-sin; +sin; 0...] x2
    wot = P("wot", (C, C_OUT), BF16)         # wo.T rows permuted to gathered order
    bob = P("bob", (128, C_OUT), F32)        # bo broadcast to all partitions
    out = P("out", (TSL, C_OUT), F32, out=True)   # transposed (t, o); host fixes

    # one AllGather unit per (pair, head): 4 units of [64, T] input each, so
    # the tail collective carries half the mesh wire of a full-pair gather
    ag_in = nc.dram_tensor("ag_in", [NP * 2 * 64, T], BF16)
    ago = [nc.dram_tensor(f"ago{u}", [G * 64, T], BF16) for u in range(NP * 2)]

    with tile.TileContext(nc) as tc, ExitStack() as ctx:
        consts = ctx.enter_context(tc.tile_pool(name="consts", bufs=1))
        io = ctx.enter_context(tc.tile_pool(name="io", bufs=2))
        qkp = ctx.enter_context(tc.tile_pool(name="qkp", bufs=2))
        vtp = ctx.enter_context(tc.tile_pool(name="vtp", bufs=2))
        ep = ctx.enter_context(tc.tile_pool(name="ep", bufs=8))
        sm = ctx.enter_context(tc.tile_pool(name="sm", bufs=8))
        vpp = ctx.enter_context(tc.tile_pool(name="vpp", bufs=4))
        tmpp = ctx.enter_context(tc.tile_pool(name="tmpp", bufs=2))
        ob = ctx.enter_context(tc.tile_pool(name="ob", bufs=2))
        opp = ctx.enter_context(tc.tile_pool(name="opp", bufs=1))
        attn_ctx = ExitStack()
        ps = attn_ctx.enter_context(tc.tile_pool(name="ps", bufs=2, space="PSUM"))
        pso = attn_ctx.enter_context(tc.tile_pool(name="pso", bufs=1, space="PSUM"))

        cm = consts.tile([128, T], F32)
        nc.sync.dma_start(out=cm, in_=cmask[:, :])
        smt = consts.tile([128, T], F32)
        nc.sync.dma_start(out=smt, in_=smask[:, :])

        # PE warmup: back-to-back matmuls under the input-DMA shadow so the
        # HAM clock gate reaches K=8/8 (2.4 GHz) before real work starts.
        # Alternate target regions so consecutive matmuls pipeline (a WAW on
        # one region stalls the array during the drain and breaks the
        # activity-window trigger).
        warm_sb = consts.tile([128, 512], BF16, tag="warm")
        nc.vector.memset(warm_sb, 0.0)
        warm_ps = ps.tile([128, 1024], F32, tag="cps")
        for w in range(16):
            half = slice(512 * (w % 2), 512 * (w % 2) + 512)
            nc.tensor.matmul(warm_ps[:, half], warm_sb[:, 0:128], warm_sb,
                             start=True, stop=True)

        for p in range(NP):          # head pairs
            outacc = pso.tile([128, T], F32, tag="outacc")
            r0 = p * 128
            xp = io.tile([128, T], BF16, tag="xp")
            nc.sync.dma_start(out=xp, in_=xh[r0:r0 + 128, :])
            cp = io.tile([128, T], BF16, tag="cp")
            nc.sync.dma_start(out=cp, in_=chd[r0:r0 + 128, :])
            qw_t = io.tile([128, 128], BF16, tag="qw")
            nc.sync.dma_start(out=qw_t, in_=qw[r0:r0 + 128, :])
            qsw_t = io.tile([128, 128], BF16, tag="qsw")
            nc.sync.dma_start(out=qsw_t, in_=qsw[r0:r0 + 128, :])
            kw_t = io.tile([128, 128], BF16, tag="kw")
            nc.sync.dma_start(out=kw_t, in_=kw[r0:r0 + 128, :])
            ksw_t = io.tile([128, 128], BF16, tag="ksw")
            nc.sync.dma_start(out=ksw_t, in_=ksw[r0:r0 + 128, :])
            vw_t = io.tile([128, 128], BF16, tag="vw")
            nc.sync.dma_start(out=vw_t, in_=vw[r0:r0 + 128, :])
            bqm_t = sm.tile([128, 1], F32, tag="bqm")
            nc.sync.dma_start(out=bqm_t, in_=bqm[r0:r0 + 128, :])
            bqs_t = sm.tile([128, 1], F32, tag="bqs")
            nc.sync.dma_start(out=bqs_t, in_=bqs[r0:r0 + 128, :])
            bkm_t = sm.tile([128, 1], F32, tag="bkm")
            nc.sync.dma_start(out=bkm_t, in_=bkm[r0:r0 + 128, :])
            bks_t = sm.tile([128, 1], F32, tag="bks")
            nc.sync.dma_start(out=bks_t, in_=bks[r0:r0 + 128, :])
            bv_t = io.tile([128, 8, 128], F32, tag="bv")
            nc.sync.dma_start(
                out=bv_t,
                in_=bass.AP(tensor=bvp.tensor, offset=bvp.offset + p * 8 * 128,
                            ap=[[0, 128], [128, 8], [1, 128]]))

            q_pair = qkp.tile([128, T], BF16, tag="q")
            k_pair = qkp.tile([128, T], BF16, tag="k")
            for dst, wt, swt, srct, bm, bs in (
                    (q_pair, qw_t, qsw_t, xp, bqm_t, bqs_t),
                    (k_pair, kw_t, ksw_t, cp, bkm_t, bks_t)):
                for hc in range(2):
                    o0 = hc * 1024
                    hs = slice(o0, o0 + 1024)
                    pm = ps.tile([128, 1024], F32, tag="cps")
                    psw = ps.tile([128, 1024], F32, tag="cps")
                    for j in range(2):
                        js = slice(j * 512, (j + 1) * 512)
                        ss = slice(o0 + j * 512, o0 + (j + 1) * 512)
                        nc.tensor.matmul(pm[:, js], wt,
                                         srct[:, ss], start=True, stop=True)
                        nc.tensor.matmul(psw[:, js], swt,
                                         srct[:, ss], start=True, stop=True)
                    # cos part on DVE, sin part on DVE, combine on GPSIMD
                    nc.vector.scalar_tensor_tensor(
                        dst[:, hs], pm, bm, cm[:, hs], op0=ALU.add, op1=ALU.mult)
                    tmp = tmpp.tile([128, 1024], F32, tag="tmp")
                    nc.vector.scalar_tensor_tensor(
                        tmp, psw, bs, smt[:, hs], op0=ALU.add, op1=ALU.mult)
                    nc.gpsimd.tensor_add(dst[:, hs], dst[:, hs], tmp)

            # v conv, transposed (t, c) with both heads' channels on the free
            # axis via the block-diagonal weight: one N=128 matmul per t-tile.
            vts = vtp.tile([128, NT, 128], BF16, tag="vts")
            for vc in range(2):
                pv = ps.tile([128, 8, 128], F32, tag="cps")
                for i8 in range(8):
                    i = vc * 8 + i8
                    ts_ = slice(128 * i, 128 * (i + 1))
                    nc.tensor.matmul(pv[:, i8, :], cp[:, ts_],
                                     vw_t, start=True, stop=True)
                nc.vector.tensor_add(vts[:, vc * 8:(vc + 1) * 8, :], pv, bv_t)

            # attention strips, software-pipelined: strip i's scores are
            # emitted before strip i-1's PV matmuls on the PE queue.
            prev = None      # (evs, vps) of strip i-1

            def emit_pv(i, evs, vps):
                for hh in range(2):
                    for half in range(2):
                        for j in range(2):
                            s4 = half * 2 + j
                            nc.tensor.matmul(
                                outacc[hh * 64:(hh + 1) * 64,
                                       s4 * 512:(s4 + 1) * 512],
                                vps[hh], evs[(hh, half)][:, j * 512:(j + 1) * 512],
                                start=(i == 0), stop=(i == NT - 1))

            for i in range(NT):
                # HAM keepalive: wait-free zero-accumulate bursts (outacc +=
                # 0*x, exact) keep the PE duty cycle high enough to hold the
                # clock gate at K=8/8 through the Act-bound strip phase.
                # Alternating banks so the burst streams gap-free; 16 matmuls
                # span a full free-running 4096-cycle activity window.
                if i % 8 == 0:
                    for w in range(16):
                        cs = slice(512 * (w % 2), 512 * (w % 2) + 512)
                        nc.tensor.matmul(outacc[0:64, cs],
                                         warm_sb[:, 0:64], warm_sb,
                                         start=False, stop=False,
                                         skip_group_check=True)
                tsl_ = slice(128 * i, 128 * (i + 1))
                evs = {}
                accs = {}
                for hh in range(2):        # hh-major: one weight load per hh
                    hr = slice(hh * 64, (hh + 1) * 64)
                    for half in range(2):
                        sp = ps.tile([128, 1024], F32, tag="cps")
                        for j in range(2):
                            s0 = half * 1024 + j * 512
                            nc.tensor.matmul(
                                sp[:, j * 512:(j + 1) * 512],
                                q_pair[hr, tsl_], k_pair[hr, s0:s0 + 512],
                                start=True, stop=True)
                        e = ep.tile([128, 1024], BF16, tag="E")
                        acc = sm.tile([128, 1], F32, tag="acc")
                        nc.scalar.activation(e, sp, AF.Exp, scale=SC,
                                             accum_out=acc)
                        evs[(hh, half)] = e
                        accs[(hh, half)] = acc
                vps = {}
                for hh in range(2):
                    den = sm.tile([128, 1], F32, tag="den")
                    nc.vector.tensor_add(den, accs[(hh, 0)], accs[(hh, 1)])
                    rec = sm.tile([128, 1], F32, tag="rec")
                    nc.vector.reciprocal(rec, den)
                    vp = vpp.tile([128, CK], BF16, tag="vp")
                    nc.vector.tensor_scalar_mul(vp, vts[:, i, hh * 64:(hh + 1) * 64], rec)
                    vps[hh] = vp
                if prev is not None:
                    emit_pv(i - 1, *prev)
                prev = (evs, vps)
            emit_pv(NT - 1, *prev)

            # PSUM drain per head: head 0 on the scalar engine (idle once the
            # exps are done) so its AllGather fires first; head 1 on DVE in
            # parallel. Each head gathers independently.
            osb = ob.tile([128, T], BF16, tag="osb")
            nc.scalar.copy(osb[0:64, :], outacc[0:64, :])
            nc.sync.dma_start(out=ag_in[p * 128:p * 128 + 64, :],
                              in_=osb[0:64, :])
            nc.gpsimd.collective_compute(
                "AllGather", ALU.bypass,
                ins=[ag_in[p * 128:p * 128 + 64, :]], outs=[ago[2 * p][:]],
                replica_groups=GROUPS)
            nc.vector.tensor_copy(osb[64:128, :], outacc[64:128, :])
            nc.sync.dma_start(out=ag_in[p * 128 + 64:p * 128 + 128, :],
                              in_=osb[64:128, :])
            nc.gpsimd.collective_compute(
                "AllGather", ALU.bypass,
                ins=[ag_in[p * 128 + 64:p * 128 + 128, :]],
                outs=[ago[2 * p + 1][:]],
                replica_groups=GROUPS)

        attn_ctx.close()

        # o-proj on this core's T slice (slice index = rank within group),
        # transposed: out_T[t, o] accumulated over 8 gathered k-chunks, so
        # the weight (lhsT) is the activation slice and reloads only per
        # (k, t-chunk). Pair-0 chunks overlap pair-1's AllGather wire.
        bo_t = consts.tile([128, C_OUT], F32)
        nc.sync.dma_start(out=bo_t, in_=bob[:, :])
        wot_t = []
        for k in range(8):
            w = consts.tile([128, C_OUT], BF16, tag=f"wot{k}")
            nc.sync.dma_start(out=w, in_=wot[128 * k:128 * (k + 1), :])
            wot_t.append(w)

        g = nc.sync.cc_rank(GROUPS)
        rhs_t = []
        for k in range(8):
            rt = opp.tile([128, TSL], BF16, tag=f"rhs{k}")
            src_t = ago[k // 2]
            rr = (k % 2) * 128
            nc.sync.dma_start(
                out=rt, in_=src_t[rr:rr + 128, bass.ts(g, TSL)])
            rhs_t.append(rt)
        with tc.tile_pool(name="opps", bufs=1, space="PSUM") as opps:
            ppm = [opps.tile([128, C_OUT], F32, tag=f"op{m}", name=f"op{m}")
                   for m in range(4)]
            for k in range(8):
                if k == 4:
                    # re-warm the PE after the AllGather idle window
                    for w in range(16):
                        nc.tensor.matmul(ppm[w % 2][:, 0:512],
                                         warm_sb[:, 0:128], warm_sb,
                                         start=False, stop=False,
                                         skip_group_check=True)
                for tcn in range(4):
                    lw = rhs_t[k][:, 128 * tcn:128 * (tcn + 1)]
                    for j in range(2):
                        nc.tensor.matmul(
                            ppm[tcn][:, 512 * j:512 * (j + 1)],
                            lw, wot_t[k][:, 512 * j:512 * (j + 1)],
                            start=(k == 0), stop=(k == 7))
            for tcn in range(4):
                ot = opp.tile([128, C_OUT], F32, tag="ot")
                nc.vector.tensor_add(ot, ppm[tcn], bo_t)
                nc.sync.dma_start(out=out[128 * tcn:128 * (tcn + 1), :], in_=ot)

    return nc


_NC_CACHE = {}


def _get_nc():
    if "nc" not in _NC_CACHE:
        _NC_CACHE["nc"] = build_nc()
    return _NC_CACHE["nc"]


def _host_consts():
    if "consts" in _NC_CACHE:
        return _NC_CACHE["consts"]
    inv_freq = (1.0 / (10000.0 ** (np.arange(HALF, dtype=np.float32) / HALF))).astype(np.float32)
    pos = np.arange(T, dtype=np.float32)
    ang = inv_freq[:, None] * pos[None, :]          # (16, T)
    cos = np.cos(ang).astype(np.float32)
    sin = np.sin(ang).astype(np.float32)
    cm64 = np.ones((CK, T), np.float32)
    cm64[0:HALF] = cos
    cm64[HALF:ROT] = cos
    sm64 = np.zeros((CK, T), np.float32)
    sm64[0:HALF] = -sin
    sm64[HALF:ROT] = sin
    cmask = np.tile(cm64, (2, 1))                   # (128, T)
    smask = np.tile(sm64, (2, 1))
    _NC_CACHE["consts"] = (cmask, smask)
    return _NC_CACHE["consts"]


PERM = np.concatenate([np.arange(HALF, ROT), np.arange(0, HALF)])


def kernel(x, c, attn_mask, wq, bq, wk, bk, wv, bv, wo, bo):
    x = np.asarray(x, np.float32)
    c = np.asarray(c, np.float32)
    wq = np.asarray(wq, np.float32)
    bq = np.asarray(bq, np.float32)
    wk = np.asarray(wk, np.float32)
    bk = np.asarray(bk, np.float32)
    wv = np.asarray(wv, np.float32)
    bv = np.asarray(bv, np.float32)
    wo = np.asarray(wo, np.float32)
    bo = np.asarray(bo, np.float32)

    cmask, smask = _host_consts()
    # permute wo.T rows to the gathered channel order:
    # virtual i -> unit u = i//256 (= pair*2 + head), rank r = (i%256)//64
    # global channel = (4r + 2*(u//2) + u%2)*64 + i%64
    gi = np.arange(C)
    gperm = (4 * ((gi % 256) // 64) + 2 * (gi // 512) + (gi // 256) % 2) * 64 + gi % 64
    wot = np.ascontiguousarray(wo.T[gperm]).astype(ml_dtypes.bfloat16)
    bob = np.ascontiguousarray(np.broadcast_to(bo[None, :], (128, C_OUT)))

    in_maps = []
    for r in range(8):
        b, g = divmod(r, G)
        qw_a = np.zeros((NP * 128, 128), np.float32)
        qsw_a = np.zeros((NP * 128, 128), np.float32)
        kw_a = np.zeros((NP * 128, 128), np.float32)
        ksw_a = np.zeros((NP * 128, 128), np.float32)
        vw_a = np.zeros((NP * 128, 128), np.float32)
        bqm_a = np.zeros((NP * 128, 1), np.float32)
        bqs_a = np.zeros((NP * 128, 1), np.float32)
        bkm_a = np.zeros((NP * 128, 1), np.float32)
        bks_a = np.zeros((NP * 128, 1), np.float32)
        bvp_a = np.zeros((NP, 8 * 128), np.float32)
        for p in range(NP):
            hA = HPC * g + 2 * p
            hB = hA + 1
            for s_, h_ in ((0, hA), (64, hB)):
                rs = slice(p * 128 + s_, p * 128 + s_ + 64)
                qw_a[rs, s_:s_ + 64] = wq[h_].T
                qsw_a[rs, s_:s_ + ROT] = wq[h_][PERM].T
                kw_a[rs, s_:s_ + 64] = wk[h_].T
                ksw_a[rs, s_:s_ + ROT] = wk[h_][PERM].T
                vw_a[rs, s_:s_ + 64] = wv[h_].T
                bqm_a[p * 128 + s_:p * 128 + s_ + 64, 0] = bq[h_ * CK:(h_ + 1) * CK]
                bkm_a[p * 128 + s_:p * 128 + s_ + 64, 0] = bk[h_ * CK:(h_ + 1) * CK]
                bqs_a[p * 128 + s_:p * 128 + s_ + ROT, 0] = \
                    bq[h_ * CK:(h_ + 1) * CK][PERM]
                bks_a[p * 128 + s_:p * 128 + s_ + ROT, 0] = \
                    bk[h_ * CK:(h_ + 1) * CK][PERM]
            bvp_a[p] = np.tile(np.concatenate([bv[hA * CK:(hA + 1) * CK],
                                               bv[hB * CK:(hB + 1) * CK]]), 8)
        ch0 = 256 * g
        bf = ml_dtypes.bfloat16
        in_maps.append({
            "xh": np.ascontiguousarray(x[b, ch0:ch0 + 256, :]).astype(bf),
            "ch": np.ascontiguousarray(c[b, ch0:ch0 + 256, :]).astype(bf),
            "qw": qw_a.astype(bf), "qsw": qsw_a.astype(bf), "kw": kw_a.astype(bf),
            "ksw": ksw_a.astype(bf), "vw": vw_a.astype(bf),
            "bqm": bqm_a, "bqs": bqs_a, "bkm": bkm_a, "bks": bks_a,
            "bvp": bvp_a,
            "cmask": cmask, "smask": smask,
            "wot": wot, "bob": bob,
        })

    global _LAST_IN_MAPS
    _LAST_IN_MAPS = in_maps
    nc = _get_nc()
    res = run_bass_kernel_spmd(nc, in_maps, core_ids=list(range(8))).results

    outf = np.empty((B, C_OUT, T), np.float32)
    for r in range(8):
        b, g = divmod(r, G)
        outf[b, :, TSL * g:TSL * (g + 1)] = res[r]["out"].T
    return outf


# revision 30
# speedup vs baseline: 1.0898x; 1.0898x over previous
"""Trainium2 Bass kernel for nn_MultiHeadAttention_42923903156587.

Sharding: 8 cores = 2 batches x 4 head-groups. Core (b, g) computes attention
for batch b, heads [4g, 4g+4), processed as two head PAIRS stacked on the
128-partition axis (head A on partitions 0:64, head B on 64:128):

- q/k convs: per-pair stacked weights; A and B matmuls are row+col tiled
  (tile_position derived from base partitions) so they run concurrently.
  RoPE is applied by full-128-lane ops: (conv+bias)*cos_mask +
  (swap_conv+bias)*sin_mask, where the "swap" channels come from extra
  permuted-weight matmuls and the masks zero the non-rotated rows.
  The two mask multiplies run on DVE; the combining add runs on GPSIMD.
- v is produced transposed (t, c) directly by the conv (lhsT = c-activation
  tile), so the softmax 1/denom (per t) is a per-partition tensor_scalar.
- scores/out matmuls run in bf16 (inputs rounded; fp32 PSUM accumulate).
  Softmax skips max-subtraction (logits are O(1)); row sums come free from
  the exp ACTIVATE accum_out.
- The strip loop is software-pipelined: strip i's score matmuls are emitted
  BEFORE strip i-1's PV matmuls so the in-order PE queue never head-of-line
  blocks on the exp (Act engine) of the current strip. Score tiles are
  ordered hh-major so each strip needs only 2 weight loads.
- out = v @ attn accumulates in one (128, T) PSUM region, head A in
  partitions 0:64, head B in 64:128 via col tile_position. The PSUM->SBUF
  drain runs on the scalar engine (idle at pair end).
- An AllGather per head-pair (overlapped with the next pair's compute)
  assembles all heads; each core then runs the full conv_o on its quarter
  of the time axis (selected with cc_rank), transposed (out rows = t) so
  weight loads halve; wo rows host-permuted to match the gathered layout,
  output transposed back on the host.
"""
import math
from contextlib import ExitStack

import ml_dtypes
import numpy as np

import concourse.bass as bass
import concourse.mybir as mybir
import concourse.tile as tile
from concourse.bass_utils import run_bass_kernel_spmd

# ---------------------------------------------------------------------------
# Workaround for this walrus build: at most ONE embedded sync-wait per TPB
# instruction is accepted. Split multi-wait instructions into single-wait NOPs.
# ---------------------------------------------------------------------------
from concourse.tile import TileContext, ScopedClock

_orig_lower = TileContext._lower_ordered_insts
_uid = [0]


def _mknop(engine, waits):
    _uid[0] += 1
    n = mybir.InstNoOp(name=f"I-waitsplit-{_uid[0]}", bass_nofuse=True)
    n.engine = engine
    n.sync_info = mybir.SyncInfo(on_wait=list(waits), on_update=[])
    return n


def _split_waits_in_list(insts):
    out = []
    for inst in insts:
        si = getattr(inst, "sync_info", None)
        if si is not None and si.on_wait and len(si.on_wait) > 1:
            waits = list(si.on_wait)
            for w in waits[:-1]:
                out.append(_mknop(inst.engine, [w]))
            inst.sync_info = mybir.SyncInfo(
                on_wait=[waits[-1]],
                on_update=list(si.on_update) if si.on_update else [],
            )
        out.append(inst)
    return out


def _patched_lower(self, ordered):
    for name in list(ordered.keys()):
        ordered[name] = _split_waits_in_list(ordered[name])
    return _orig_lower(self, ordered)


def _patched_drain_and_barrier(self, tick_clock, wait_clock):
    nc = self.nc
    carrier = nc.sync.nop(nofuse=True)
    wait_clock.add_sem_waits(carrier.ins, ScopedClock({None: tick_clock.global_clock}))
    si = carrier.ins.sync_info
    waits = list(si.on_wait) if si is not None and si.on_wait else []
    if len(waits) > 1:
        carrier.ins.sync_info = mybir.SyncInfo(
            on_wait=waits[:1],
            on_update=list(si.on_update) if si.on_update else [],
        )
        for w in waits[1:]:
            extra = nc.sync.nop(nofuse=True)
            extra.ins.sync_info = mybir.SyncInfo(on_wait=[w], on_update=[])
    nc.sync.drain()

    nc.all_engine_barrier()
    assert self.sems is not None
    popped = nc._tile_sem_poison_stack.pop()
    assert popped is self._sem_poison
    nc.clear_and_free_semaphores(list(self.sems.allocated().values()))
    nc.all_engine_barrier()


TileContext._lower_ordered_insts = _patched_lower
TileContext._drain_and_barrier = _patched_drain_and_barrier

# ---------------------------------------------------------------------------

F32 = mybir.dt.float32
F32R = mybir.dt.float32r
BF16 = mybir.dt.bfloat16
AF = mybir.ActivationFunctionType
ALU = mybir.AluOpType

B, C, T, H = 2, 1024, 2048, 16
CK = 64          # head dim
ROT = 32         # rotated head dims
HALF = 16
C_OUT = 1024
HPC = 4          # heads per core
NP = 2           # head pairs per core
G = 4            # cores per batch group
TSL = T // G     # 512: o-proj T slice per core
NT = T // 128    # 16 t-tiles per head
SC = 1.0 / math.sqrt(CK)
GROUPS = [[0, 1, 2, 3], [4, 5, 6, 7]]


def build_nc():
    nc = bass.Bass("TRN2", target_bir_lowering=False, debug=False, num_devices=8)

    def P(name, shape, dtype, out=False):
        return nc.dram_tensor(name, list(shape), dtype,
                              kind="ExternalOutput" if out else "ExternalInput").ap()

    xh = P("xh", (HPC * CK, T), BF16)        # x channels, pair p rows 128p:128p+128
    chd = P("ch", (HPC * CK, T), BF16)
    # block-diagonal per-pair conv weights: one K=128 matmul per chunk
    qw = P("qw", (NP * 128, 128), BF16)      # blockdiag(wqA.T, wqB.T)
    qsw = P("qsw", (NP * 128, 128), BF16)    # perm weights in cols 0:32 / 64:96
    kw = P("kw", (NP * 128, 128), BF16)
    ksw = P("ksw", (NP * 128, 128), BF16)
    vw = P("vw", (NP * 128, 128), BF16)      # blockdiag(wvA.T, wvB.T)
    bqm = P("bqm", (NP * 128, 1), F32)       # [bqA; bqB]
    bqs = P("bqs", (NP * 128, 1), F32)       # [bqA[PERM]; 0; bqB[PERM]; 0]
    bkm = P("bkm", (NP * 128, 1), F32)
    bks = P("bks", (NP * 128, 1), F32)
    bvp = P("bvp", (NP, 8 * 128), F32)       # per pair: tile([bvA; bvB], 8)
    cmask = P("cmask", (128, T), F32)        # [cos; cos; 1; 1] x2
    smask = P("smask", (128, T), F32)        # [-sin; +sin; 0...] x2
    wot = P("wot", (C, C_OUT), BF16)         # wo.T rows permuted to gathered order
    bob = P("bob", (128, C_OUT), F32)        # bo broadcast to all partitions
    out = P("out", (TSL, C_OUT), F32, out=True)   # transposed (t, o); host fixes

    ag_in = nc.dram_tensor("ag_in", [NP * 128, T], BF16)
    ago = [nc.dram_tensor(f"ago{p}", [G * 128, T], BF16) for p in range(NP)]

    with tile.TileContext(nc) as tc, ExitStack() as ctx:
        consts = ctx.enter_context(tc.tile_pool(name="consts", bufs=1))
        io = ctx.enter_context(tc.tile_pool(name="io", bufs=2))
        qkp = ctx.enter_context(tc.tile_pool(name="qkp", bufs=2))
        vtp = ctx.enter_context(tc.tile_pool(name="vtp", bufs=2))
        ep = ctx.enter_context(tc.tile_pool(name="ep", bufs=8))
        sm = ctx.enter_context(tc.tile_pool(name="sm", bufs=8))
        vpp = ctx.enter_context(tc.tile_pool(name="vpp", bufs=4))
        tmpp = ctx.enter_context(tc.tile_pool(name="tmpp", bufs=2))
        ob = ctx.enter_context(tc.tile_pool(name="ob", bufs=2))
        opp = ctx.enter_context(tc.tile_pool(name="opp", bufs=1))
        attn_ctx = ExitStack()
        ps = attn_ctx.enter_context(tc.tile_pool(name="ps", bufs=2, space="PSUM"))
        pso = attn_ctx.enter_context(tc.tile_pool(name="pso", bufs=1, space="PSUM"))

        cm = consts.tile([128, T], F32)
        nc.sync.dma_start(out=cm, in_=cmask[:, :])
        smt = consts.tile([128, T], F32)
        nc.sync.dma_start(out=smt, in_=smask[:, :])

        # PE warmup: back-to-back matmuls under the input-DMA shadow so the
        # HAM clock gate reaches K=8/8 (2.4 GHz) before real work starts.
        # Alternate target regions so consecutive matmuls pipeline (a WAW on
        # one region stalls the array during the drain and breaks the
        # activity-window trigger).
        warm_sb = consts.tile([128, 512], BF16, tag="warm")
        nc.vector.memset(warm_sb, 0.0)
        warm_ps = ps.tile([128, 1024], F32, tag="cps")
        for w in range(16):
            half = slice(512 * (w % 2), 512 * (w % 2) + 512)
            nc.tensor.matmul(warm_ps[:, half], warm_sb[:, 0:128], warm_sb,
                             start=True, stop=True)

        for p in range(NP):          # head pairs
            outacc = pso.tile([128, T], F32, tag="outacc")
            r0 = p * 128
            xp = io.tile([128, T], BF16, tag="xp")
            nc.sync.dma_start(out=xp, in_=xh[r0:r0 + 128, :])
            cp = io.tile([128, T], BF16, tag="cp")
            nc.sync.dma_start(out=cp, in_=chd[r0:r0 + 128, :])
            qw_t = io.tile([128, 128], BF16, tag="qw")
            nc.sync.dma_start(out=qw_t, in_=qw[r0:r0 + 128, :])
            qsw_t = io.tile([128, 128], BF16, tag="qsw")
            nc.sync.dma_start(out=qsw_t, in_=qsw[r0:r0 + 128, :])
            kw_t = io.tile([128, 128], BF16, tag="kw")
            nc.sync.dma_start(out=kw_t, in_=kw[r0:r0 + 128, :])
            ksw_t = io.tile([128, 128], BF16, tag="ksw")
            nc.sync.dma_start(out=ksw_t, in_=ksw[r0:r0 + 128, :])
            vw_t = io.tile([128, 128], BF16, tag="vw")
            nc.sync.dma_start(out=vw_t, in_=vw[r0:r0 + 128, :])
            bqm_t = sm.tile([128, 1], F32, tag="bqm")
            nc.sync.dma_start(out=bqm_t, in_=bqm[r0:r0 + 128, :])
            bqs_t = sm.tile([128, 1], F32, tag="bqs")
            nc.sync.dma_start(out=bqs_t, in_=bqs[r0:r0 + 128, :])
            bkm_t = sm.tile([128, 1], F32, tag="bkm")
            nc.sync.dma_start(out=bkm_t, in_=bkm[r0:r0 + 128, :])
            bks_t = sm.tile([128, 1], F32, tag="bks")
            nc.sync.dma_start(out=bks_t, in_=bks[r0:r0 + 128, :])
            bv_t = io.tile([128, 8, 128], F32, tag="bv")
            nc.sync.dma_start(
                out=bv_t,
                in_=bass.AP(tensor=bvp.tensor, offset=bvp.offset + p * 8 * 128,
                            ap=[[0, 128], [128, 8], [1, 128]]))

            q_pair = qkp.tile([128, T], BF16, tag="q")
            k_pair = qkp.tile([128, T], BF16, tag="k")
            for dst, wt, swt, srct, bm, bs in (
                    (q_pair, qw_t, qsw_t, xp, bqm_t, bqs_t),
                    (k_pair, kw_t, ksw_t, cp, bkm_t, bks_t)):
                for hc in range(2):
                    o0 = hc * 1024
                    hs = slice(o0, o0 + 1024)
                    pm = ps.tile([128, 1024], F32, tag="cps")
                    psw = ps.tile([128, 1024], F32, tag="cps")
                    for j in range(2):
                        js = slice(j * 512, (j + 1) * 512)
                        ss = slice(o0 + j * 512, o0 + (j + 1) * 512)
                        nc.tensor.matmul(pm[:, js], wt,
                                         srct[:, ss], start=True, stop=True)
                        nc.tensor.matmul(psw[:, js], swt,
                                         srct[:, ss], start=True, stop=True)
                    # cos part on DVE, sin part on DVE, combine on GPSIMD
                    nc.vector.scalar_tensor_tensor(
                        dst[:, hs], pm, bm, cm[:, hs], op0=ALU.add, op1=ALU.mult)
                    tmp = tmpp.tile([128, 1024], F32, tag="tmp")
                    nc.vector.scalar_tensor_tensor(
                        tmp, psw, bs, smt[:, hs], op0=ALU.add, op1=ALU.mult)
                    nc.gpsimd.tensor_add(dst[:, hs], dst[:, hs], tmp)

            # v conv, transposed (t, c) with both heads' channels on the free
            # axis via the block-diagonal weight: one N=128 matmul per t-tile.
            vts = vtp.tile([128, NT, 128], BF16, tag="vts")
            for vc in range(2):
                pv = ps.tile([128, 8, 128], F32, tag="cps")
                for i8 in range(8):
                    i = vc * 8 + i8
                    ts_ = slice(128 * i, 128 * (i + 1))
                    nc.tensor.matmul(pv[:, i8, :], cp[:, ts_],
                                     vw_t, start=True, stop=True)
                nc.vector.tensor_add(vts[:, vc * 8:(vc + 1) * 8, :], pv, bv_t)

            # attention strips, software-pipelined: strip i's scores are
            # emitted before strip i-1's PV matmuls on the PE queue.
            prev = None      # (evs, vps) of strip i-1

            def emit_pv(i, evs, vps):
                for hh in range(2):
                    for half in range(2):
                        for j in range(2):
                            s4 = half * 2 + j
                            nc.tensor.matmul(
                                outacc[hh * 64:(hh + 1) * 64,
                                       s4 * 512:(s4 + 1) * 512],
                                vps[hh], evs[(hh, half)][:, j * 512:(j + 1) * 512],
                                start=(i == 0), stop=(i == NT - 1))

            for i in range(NT):
                # HAM keepalive: wait-free zero-accumulate bursts (outacc +=
                # 0*x, exact) keep the PE duty cycle high enough to hold the
                # clock gate at K=8/8 through the Act-bound strip phase.
                # Alternating banks so the burst streams gap-free; 16 matmuls
                # span a full free-running 4096-cycle activity window.
                if i % 4 == 0:
                    for w in range(12):
                        cs = slice(512 * (w % 2), 512 * (w % 2) + 512)
                        nc.tensor.matmul(outacc[0:64, cs],
                                         warm_sb[:, 0:64], warm_sb,
                                         start=False, stop=False,
                                         skip_group_check=True)
                tsl_ = slice(128 * i, 128 * (i + 1))
                evs = {}
                accs = {}
                for hh in range(2):        # hh-major: one weight load per hh
                    hr = slice(hh * 64, (hh + 1) * 64)
                    for half in range(2):
                        sp = ps.tile([128, 1024], F32, tag="cps")
                        for j in range(2):
                            s0 = half * 1024 + j * 512
                            nc.tensor.matmul(
                                sp[:, j * 512:(j + 1) * 512],
                                q_pair[hr, tsl_], k_pair[hr, s0:s0 + 512],
                                start=True, stop=True)
                        e = ep.tile([128, 1024], BF16, tag="E")
                        acc = sm.tile([128, 1], F32, tag="acc")
                        nc.scalar.activation(e, sp, AF.Exp, scale=SC,
                                             accum_out=acc)
                        evs[(hh, half)] = e
                        accs[(hh, half)] = acc
                vps = {}
                for hh in range(2):
                    den = sm.tile([128, 1], F32, tag="den")
                    nc.vector.tensor_add(den, accs[(hh, 0)], accs[(hh, 1)])
                    rec = sm.tile([128, 1], F32, tag="rec")
                    nc.vector.reciprocal(rec, den)
                    vp = vpp.tile([128, CK], BF16, tag="vp")
                    nc.vector.tensor_scalar_mul(vp, vts[:, i, hh * 64:(hh + 1) * 64], rec)
                    vps[hh] = vp
                if prev is not None:
                    emit_pv(i - 1, *prev)
                prev = (evs, vps)
            emit_pv(NT - 1, *prev)

            # PSUM drain split across the scalar engine (idle once the exps
            # are done) and DVE so the halves run in parallel.
            osb = ob.tile([128, T], BF16, tag="osb")
            nc.scalar.copy(osb[0:64, :], outacc[0:64, :])
            nc.vector.tensor_copy(osb[64:128, :], outacc[64:128, :])
            nc.sync.dma_start(out=ag_in[p * 128:(p + 1) * 128, :], in_=osb)
            nc.gpsimd.collective_compute(
                "AllGather", ALU.bypass,
                ins=[ag_in[p * 128:(p + 1) * 128, :]], outs=[ago[p][:]],
                replica_groups=GROUPS)

        attn_ctx.close()

        # o-proj on this core's T slice (slice index = rank within group),
        # transposed: out_T[t, o] accumulated over 8 gathered k-chunks, so
        # the weight (lhsT) is the activation slice and reloads only per
        # (k, t-chunk). Pair-0 chunks overlap pair-1's AllGather wire.
        bo_t = consts.tile([128, C_OUT], F32)
        nc.sync.dma_start(out=bo_t, in_=bob[:, :])
        wot_t = []
        for k in range(8):
            w = consts.tile([128, C_OUT], BF16, tag=f"wot{k}")
            nc.sync.dma_start(out=w, in_=wot[128 * k:128 * (k + 1), :])
            wot_t.append(w)

        g = nc.sync.cc_rank(GROUPS)
        rhs_t = []
        for k in range(8):
            rt = opp.tile([128, TSL], BF16, tag=f"rhs{k}")
            src_t = ago[k // 4]
            rr = (k % 4) * 128
            nc.sync.dma_start(
                out=rt, in_=src_t[rr:rr + 128, bass.ts(g, TSL)])
            rhs_t.append(rt)
        with tc.tile_pool(name="opps", bufs=1, space="PSUM") as opps:
            ppm = [opps.tile([128, C_OUT], F32, tag=f"op{m}", name=f"op{m}")
                   for m in range(4)]
            for k in range(4):
                for tcn in range(4):
                    lw = rhs_t[k][:, 128 * tcn:128 * (tcn + 1)]
                    for j in range(2):
                        nc.tensor.matmul(
                            ppm[tcn][:, 512 * j:512 * (j + 1)],
                            lw, wot_t[k][:, 512 * j:512 * (j + 1)],
                            start=(k == 0), stop=False)
            # re-warm the PE after the AllGather idle window
            for w in range(12):
                nc.tensor.matmul(ppm[w % 2][:, 0:512],
                                 warm_sb[:, 0:128], warm_sb,
                                 start=False, stop=False,
                                 skip_group_check=True)
            # t-chunk-outer for the post-AllGather half so each chunk's bias
            # add + output DMA overlaps the remaining chunks' matmuls
            for tcn in range(4):
                for k in range(4, 8):
                    lw = rhs_t[k][:, 128 * tcn:128 * (tcn + 1)]
                    for j in range(2):
                        nc.tensor.matmul(
                            ppm[tcn][:, 512 * j:512 * (j + 1)],
                            lw, wot_t[k][:, 512 * j:512 * (j + 1)],
                            start=False, stop=(k == 7))
                ot = opp.tile([128, C_OUT], F32, tag="ot")
                nc.vector.tensor_add(ot, ppm[tcn], bo_t)
                nc.sync.dma_start(out=out[128 * tcn:128 * (tcn + 1), :], in_=ot)

    return nc


_NC_CACHE = {}


def _get_nc():
    if "nc" not in _NC_CACHE:
        _NC_CACHE["nc"] = build_nc()
    return _NC_CACHE["nc"]


def _host_consts():
    if "consts" in _NC_CACHE:
        return _NC_CACHE["consts"]
    inv_freq = (1.0 / (10000.0 ** (np.arange(HALF, dtype=np.float32) / HALF))).astype(np.float32)
    pos = np.arange(T, dtype=np.float32)
    ang = inv_freq[:, None] * pos[None, :]          # (16, T)
    cos = np.cos(ang).astype(np.float32)
    sin = np.sin(ang).astype(np.float32)
    cm64 = np.ones((CK, T), np.float32)
    cm64[0:HALF] = cos
    cm64[HALF:ROT] = cos
    sm64 = np.zeros((CK, T), np.float32)
    sm64[0:HALF] = -sin
    sm64[HALF:ROT] = sin
    cmask = np.tile(cm64, (2, 1))                   # (128, T)
    smask = np.tile(sm64, (2, 1))
    _NC_CACHE["consts"] = (cmask, smask)
    return _NC_CACHE["consts"]


PERM = np.concatenate([np.arange(HALF, ROT), np.arange(0, HALF)])


def kernel(x, c, attn_mask, wq, bq, wk, bk, wv, bv, wo, bo):
    x = np.asarray(x, np.float32)
    c = np.asarray(c, np.float32)
    wq = np.asarray(wq, np.float32)
    bq = np.asarray(bq, np.float32)
    wk = np.asarray(wk, np.float32)
    bk = np.asarray(bk, np.float32)
    wv = np.asarray(wv, np.float32)
    bv = np.asarray(bv, np.float32)
    wo = np.asarray(wo, np.float32)
    bo = np.asarray(bo, np.float32)

    cmask, smask = _host_consts()
    # permute wo.T rows to the gathered channel order:
    # virtual i -> pair p = i//512, rank r = (i%512)//128, w = i%128
    # global channel = (4r + 2p + w//64)*64 + w%64
    gi = np.arange(C)
    gperm = (4 * ((gi % 512) // 128) + 2 * (gi // 512) + (gi % 128) // 64) * 64 + gi % 64
    wot = np.ascontiguousarray(wo.T[gperm]).astype(ml_dtypes.bfloat16)
    bob = np.ascontiguousarray(np.broadcast_to(bo[None, :], (128, C_OUT)))

    in_maps = []
    for r in range(8):
        b, g = divmod(r, G)
        qw_a = np.zeros((NP * 128, 128), np.float32)
        qsw_a = np.zeros((NP * 128, 128), np.float32)
        kw_a = np.zeros((NP * 128, 128), np.float32)
        ksw_a = np.zeros((NP * 128, 128), np.float32)
        vw_a = np.zeros((NP * 128, 128), np.float32)
        bqm_a = np.zeros((NP * 128, 1), np.float32)
        bqs_a = np.zeros((NP * 128, 1), np.float32)
        bkm_a = np.zeros((NP * 128, 1), np.float32)
        bks_a = np.zeros((NP * 128, 1), np.float32)
        bvp_a = np.zeros((NP, 8 * 128), np.float32)
        for p in range(NP):
            hA = HPC * g + 2 * p
            hB = hA + 1
            for s_, h_ in ((0, hA), (64, hB)):
                rs = slice(p * 128 + s_, p * 128 + s_ + 64)
                qw_a[rs, s_:s_ + 64] = wq[h_].T
                qsw_a[rs, s_:s_ + ROT] = wq[h_][PERM].T
                kw_a[rs, s_:s_ + 64] = wk[h_].T
                ksw_a[rs, s_:s_ + ROT] = wk[h_][PERM].T
                vw_a[rs, s_:s_ + 64] = wv[h_].T
                bqm_a[p * 128 + s_:p * 128 + s_ + 64, 0] = bq[h_ * CK:(h_ + 1) * CK]
                bkm_a[p * 128 + s_:p * 128 + s_ + 64, 0] = bk[h_ * CK:(h_ + 1) * CK]
                bqs_a[p * 128 + s_:p * 128 + s_ + ROT, 0] = \
                    bq[h_ * CK:(h_ + 1) * CK][PERM]
                bks_a[p * 128 + s_:p * 128 + s_ + ROT, 0] = \
                    bk[h_ * CK:(h_ + 1) * CK][PERM]
            bvp_a[p] = np.tile(np.concatenate([bv[hA * CK:(hA + 1) * CK],
                                               bv[hB * CK:(hB + 1) * CK]]), 8)
        ch0 = 256 * g
        bf = ml_dtypes.bfloat16
        in_maps.append({
            "xh": np.ascontiguousarray(x[b, ch0:ch0 + 256, :]).astype(bf),
            "ch": np.ascontiguousarray(c[b, ch0:ch0 + 256, :]).astype(bf),
            "qw": qw_a.astype(bf), "qsw": qsw_a.astype(bf), "kw": kw_a.astype(bf),
            "ksw": ksw_a.astype(bf), "vw": vw_a.astype(bf),
            "bqm": bqm_a, "bqs": bqs_a, "bkm": bkm_a, "bks": bks_a,
            "bvp": bvp_a,
            "cmask": cmask, "smask": smask,
            "wot": wot, "bob": bob,
        })

    global _LAST_IN_MAPS
    _LAST_IN_MAPS = in_maps
    nc = _get_nc()
    res = run_bass_kernel_spmd(nc, in_maps, core_ids=list(range(8))).results

    outf = np.empty((B, C_OUT, T), np.float32)
    for r in range(8):
        b, g = divmod(r, G)
        outf[b, :, TSL * g:TSL * (g + 1)] = res[r]["out"].T
    return outf
